# revision 13
# baseline (speedup 1.0000x reference)
"""Trainium2 Bass kernel for the 2-layer GRU language model.

Data-parallel over 8 NeuronCores: batch N=1024 sharded 128/core, weights
replicated. Per core, a fused per-step loop runs both GRU layers + the
FC/log-softmax/NLL tail; layer-0's input projection is pre-folded into a
64x3072 table (M0b = emb @ W_ih0.T + biases) gathered via one-hot matmuls.
All big GEMMs in bf16 (fp32 accumulate), gates/softmax in fp32/bf16 mix.
"""
import numpy as np
import ml_dtypes

import concourse.bass as bass
import concourse.mybir as mybir
import concourse.tile as tile
from concourse.bass_utils import run_bass_kernel_spmd

BF16 = mybir.dt.bfloat16
F32 = mybir.dt.float32
I32 = mybir.dt.int32
AF = mybir.ActivationFunctionType
ALU = mybir.AluOpType

N, S = 1024, 128
V, E, H = 64, 512, 1024
B = 128           # per-core batch
T = S - 1         # 127 steps
NC = 8
KH = H // 128     # 8 K-chunks of H
G3 = 3 * H        # 3072


def split_drain_waits(nc):
    """walrus here accepts only 1 sync wait per instruction; hoist extras
    onto preceding NoOps on the same engine."""
    for f in nc.m.functions:
        for blk in f.blocks:
            new_list, changed = [], False
            for inst in blk.instructions:
                si = getattr(inst, "sync_info", None)
                if (si is not None and len(si.on_wait) > 1):
                    for k, w in enumerate(si.on_wait[:-1]):
                        nop = mybir.InstNoOp(
                            name=f"{inst.name}-w{k}",
                            sync_info=mybir.SyncInfo(on_wait=[w], on_update=[]),
                            bass_nofuse=True,
                            engine=inst.engine,
                        )
                        new_list.append(nop)
                    inst.sync_info = mybir.SyncInfo(
                        on_wait=[si.on_wait[-1]], on_update=si.on_update)
                    changed = True
                new_list.append(inst)
            if changed:
                blk.instructions = new_list


def build_kernel(n_steps=T, split_drains=True, repeat=None):
    nc = bass.Bass()
    dp = nc.declare_dram_parameter
    d_x = dp("x", [B, S], F32, isOutput=False)
    d_embT = dp("embT", [128, (E // 128) * V], BF16, isOutput=False)
    d_wih0 = dp("wih0", [128, (E // 128) * G3], BF16, isOutput=False)
    d_whh0 = dp("whh0", [128, KH * G3], BF16, isOutput=False)
    d_wih1 = dp("wih1", [128, KH * G3], BF16, isOutput=False)
    d_whh1 = dp("whh1", [128, KH * G3], BF16, isOutput=False)
    d_wfc = dp("wfc", [128, KH * V], BF16, isOutput=False)
    d_brow0 = dp("brow0", [1, G3], BF16, isOutput=False)   # bih0 + bhh0(rz only)
    d_bhh0n = dp("bhh0n", [1, H], BF16, isOutput=False)
    d_brz1 = dp("brz1", [1, 2 * H], BF16, isOutput=False)  # bih1+bhh1 rz
    d_bn1 = dp("bn1", [1, H], BF16, isOutput=False)        # bih1 n-part
    d_bhh1n = dp("bhh1n", [1, H], BF16, isOutput=False)
    d_bfc = dp("bfc", [1, V], BF16, isOutput=False)
    d_lp = dp("lp", [T, B, V], F32, isOutput=True)
    d_mol = dp("mol", [B, 1], F32, isOutput=True)
    d_lsum = dp("lsum", [1, 1], F32, isOutput=True)

    with tile.TileContext(nc) as tc:
        _body(nc, tc, n_steps,
              d_x, d_embT, d_wih0, d_whh0, d_wih1, d_whh1, d_wfc,
              d_brow0, d_bhh0n, d_brz1, d_bn1, d_bhh1n, d_bfc,
              d_lp, d_mol, d_lsum, repeat=repeat)
    if split_drains:
        split_drain_waits(nc)
    return nc


def _body(nc, tc, n_steps,
          d_x, d_embT, d_wih0, d_whh0, d_wih1, d_whh1, d_wfc,
          d_brow0, d_bhh0n, d_brz1, d_bn1, d_bhh1n, d_bfc,
          d_lp, d_mol, d_lsum, repeat=None):
    from contextlib import ExitStack
    ctx = ExitStack()
    with ctx:
        # ---------- persistent pool (init pools created inline, then the
        # steady-state pools, so init scratch space is reused) ----------
        pw = ctx.enter_context(tc.tile_pool(name="pw", bufs=1))       # persistent
        if repeat is not None:
            ctx.enter_context(tc.For_i(0, repeat, 1, name="rep"))

        # ---------- persistent loads ----------
        x_sb = pw.tile([B, S], F32, tag="x")
        nc.sync.dma_start(x_sb[:], d_x[:])
        w_hh0 = pw.tile([128, KH * G3], BF16, tag="whh0")
        w_ih1 = pw.tile([128, KH * G3], BF16, tag="wih1")
        w_hh1 = pw.tile([128, KH * G3], BF16, tag="whh1")
        for k in range(KH):
            nc.sync.dma_start(w_hh0[:, bass.ts(k, G3)], d_whh0[:, bass.ts(k, G3)])
        for k in range(KH):
            nc.sync.dma_start(w_ih1[:, bass.ts(k, G3)], d_wih1[:, bass.ts(k, G3)])
        for k in range(KH):
            nc.sync.dma_start(w_hh1[:, bass.ts(k, G3)], d_whh1[:, bass.ts(k, G3)])
        w_fc = pw.tile([128, KH * V], BF16, tag="wfc")
        nc.sync.dma_start(w_fc[:], d_wfc[:])
        b_bhh0n = pw.tile([1, H], BF16, tag="bhh0n")
        nc.sync.dma_start(b_bhh0n[:], d_bhh0n[:])
        b_brz1 = pw.tile([1, 2 * H], BF16, tag="brz1")
        nc.sync.dma_start(b_brz1[:], d_brz1[:])
        b_bn1 = pw.tile([1, H], BF16, tag="bn1")
        nc.sync.dma_start(b_bn1[:], d_bn1[:])
        b_bhh1n = pw.tile([1, H], BF16, tag="bhh1n")
        nc.sync.dma_start(b_bhh1n[:], d_bhh1n[:])
        b_bfc = pw.tile([1, V], BF16, tag="bfc")
        nc.sync.dma_start(b_bfc[:], d_bfc[:])

        # ---------- constants ----------
        iota64 = pw.tile([B, V], F32, tag="iota64")
        nc.gpsimd.iota(iota64[:], pattern=[[1, V]], base=0, channel_multiplier=0, allow_small_or_imprecise_dtypes=True)
        iota_m64 = pw.tile([B, V], F32, tag="iotam64")
        nc.gpsimd.iota(iota_m64[:], pattern=[[1, V]], base=0, channel_multiplier=0, allow_small_or_imprecise_dtypes=True)
        nc.gpsimd.memset(iota_m64[:, 0:1], -1.0)  # col0 never matches -> masks tgt==0
        iota128 = pw.tile([128, 128], F32, tag="iota128")
        nc.gpsimd.iota(iota128[:], pattern=[[1, 128]], base=0, channel_multiplier=0, allow_small_or_imprecise_dtypes=True)
        iota_col = pw.tile([128, 1], F32, tag="iotacol")
        nc.gpsimd.iota(iota_col[:], pattern=[[0, 1]], base=0, channel_multiplier=1, allow_small_or_imprecise_dtypes=True)
        ident = pw.tile([128, 128], BF16, tag="ident")
        nc.vector.tensor_scalar(ident[:], iota128[:], iota_col[:, 0:1], None,
                                ALU.is_equal)
        ones_row = pw.tile([1, 128], BF16, tag="onesrow")
        nc.vector.memset(ones_row[:], 1.0)
        ones_col = pw.tile([128, 1], F32, tag="onescol")
        nc.vector.memset(ones_col[:], 1.0)

        # ---------- M0b table: emb @ W_ih0.T + brow0 ----------
        m0b = pw.tile([V, G3], BF16, tag="m0b")
        with tc.tile_pool(name="pinit", bufs=2) as pinit, \
             tc.tile_pool(name="ps_m0", bufs=6, space="PSUM") as ps_m0:
            embT = pinit.tile([128, (E // 128) * V], BF16, tag="embT", bufs=1)
            nc.sync.dma_start(embT[:], d_embT[:])
            b_brow0 = pinit.tile([1, G3], BF16, tag="brow0", bufs=1)
            nc.sync.dma_start(b_brow0[:], d_brow0[:])
            m0ps = [ps_m0.tile([128, 512], F32, tag="m0", name=f"m0ps{g}")
                    for g in range(G3 // 512)]
            for k in range(E // 128):
                wih0_k = pinit.tile([128, G3], BF16, tag="wih0k")
                nc.sync.dma_start(wih0_k[:], d_wih0[:, bass.ts(k, G3)])
                for g in range(G3 // 512):
                    ps = m0ps[g]
                    if k == 0:
                        nc.tensor.matmul(ps[0:V, :], ones_row[0:1, 0:V],
                                         b_brow0[0:1, bass.ts(g, 512)],
                                         start=True, stop=False)
                    nc.tensor.matmul(ps[0:V, :], embT[:, bass.ts(k, V)],
                                     wih0_k[:, bass.ts(g, 512)],
                                     start=False, stop=(k == E // 128 - 1))
            for g in range(G3 // 512):
                nc.vector.tensor_copy(m0b[:, bass.ts(g, 512)], m0ps[g][0:V, :])

        # ---------- steady-state pools ----------
        pstate = ctx.enter_context(tc.tile_pool(name="pstate", bufs=2))
        pgate = ctx.enter_context(tc.tile_pool(name="pgate", bufs=1))
        psoft = ctx.enter_context(tc.tile_pool(name="psoft", bufs=2))
        ps_gb = ctx.enter_context(tc.tile_pool(name="ps_gb", bufs=4, space="PSUM"))
        ps_tp = ctx.enter_context(tc.tile_pool(name="ps_tp", bufs=2, space="PSUM"))
        ps_fc = ctx.enter_context(tc.tile_pool(name="ps_fc", bufs=1, space="PSUM"))
        ps_oh = ctx.enter_context(tc.tile_pool(name="ps_oh", bufs=1, space="PSUM"))

        # ---------- states / accumulators ----------
        s0 = pw.tile([B, H], BF16, tag="s0i")
        nc.vector.memset(s0[:], 0.0)
        s1 = pw.tile([B, H], BF16, tag="s1i")
        nc.vector.memset(s1[:], 0.0)
        nll = [pw.tile([B, 1], F32, tag=f"nll{i}", name=f"nll{i}") for i in range(2)]
        nc.vector.memset(nll[0][:], 0.0)
        s0T_prev = None
        s1T_prev = None

        def gate_banks(layer, t, ohT, s_inT, s_recT):
            """Emit the 8 psum gate banks for one layer; returns list of
            (kind, psum_tile) in order r0,r1,z0,z1,hn0,hn1,xn0,xn1 slices."""
            banks = []
            if layer == 0:
                # rz banks 0..3: onehot(M0b) + recurrent
                for g in range(4):
                    ps = ps_gb.tile([128, 512], F32, tag="gb")
                    nc.tensor.matmul(ps[:], ohT[:], m0b[:, bass.ts(g, 512)],
                                     start=True, stop=(t == 0))
                    if t > 0:
                        for k in range(KH):
                            nc.tensor.matmul(
                                ps[:], s_recT[:, bass.ts(k, 128)],
                                w_hh0[:, k * G3 + g * 512:k * G3 + (g + 1) * 512],
                                start=False, stop=(k == KH - 1))
                    banks.append(ps)
                # hn banks: bhh0n + recurrent
                for g in range(2):
                    ps = ps_gb.tile([128, 512], F32, tag="gb")
                    nc.tensor.matmul(ps[:], ones_row[0:1, :],
                                     b_bhh0n[0:1, bass.ts(g, 512)],
                                     start=True, stop=(t == 0))
                    if t > 0:
                        for k in range(KH):
                            nc.tensor.matmul(
                                ps[:], s_recT[:, bass.ts(k, 128)],
                                w_hh0[:, k * G3 + 2048 + g * 512:
                                      k * G3 + 2048 + (g + 1) * 512],
                                start=False, stop=(k == KH - 1))
                    banks.append(ps)
                # xn banks: onehot(M0b n-part)
                for g in range(2):
                    ps = ps_gb.tile([128, 512], F32, tag="gb")
                    nc.tensor.matmul(ps[:], ohT[:],
                                     m0b[:, 2048 + g * 512:2048 + (g + 1) * 512],
                                     start=True, stop=True)
                    banks.append(ps)
            else:
                # rz banks: brz1 + gh1(recurrent, skip at t=0) + gx1(input)
                for g in range(4):
                    ps = ps_gb.tile([128, 512], F32, tag="gb")
                    nc.tensor.matmul(ps[:], ones_row[0:1, :],
                                     b_brz1[0:1, bass.ts(g, 512)],
                                     start=True, stop=False)
                    if t > 0:
                        for k in range(KH):
                            nc.tensor.matmul(
                                ps[:], s_recT[:, bass.ts(k, 128)],
                                w_hh1[:, k * G3 + g * 512:k * G3 + (g + 1) * 512],
                                start=False, stop=False)
                    for k in range(KH):
                        nc.tensor.matmul(
                            ps[:], s_inT[:, bass.ts(k, 128)],
                            w_ih1[:, k * G3 + g * 512:k * G3 + (g + 1) * 512],
                            start=False, stop=(k == KH - 1))
                    banks.append(ps)
                # hn banks: bhh1n + gh1 recurrent
                for g in range(2):
                    ps = ps_gb.tile([128, 512], F32, tag="gb")
                    nc.tensor.matmul(ps[:], ones_row[0:1, :],
                                     b_bhh1n[0:1, bass.ts(g, 512)],
                                     start=True, stop=(t == 0))
                    if t > 0:
                        for k in range(KH):
                            nc.tensor.matmul(
                                ps[:], s_recT[:, bass.ts(k, 128)],
                                w_hh1[:, k * G3 + 2048 + g * 512:
                                      k * G3 + 2048 + (g + 1) * 512],
                                start=False, stop=(k == KH - 1))
                    banks.append(ps)
                # xn banks: bn1 + gx1 n-part
                for g in range(2):
                    ps = ps_gb.tile([128, 512], F32, tag="gb")
                    nc.tensor.matmul(ps[:], ones_row[0:1, :],
                                     b_bn1[0:1, bass.ts(g, 512)],
                                     start=True, stop=False)
                    for k in range(KH):
                        nc.tensor.matmul(
                            ps[:], s_inT[:, bass.ts(k, 128)],
                            w_ih1[:, k * G3 + 2048 + g * 512:
                                  k * G3 + 2048 + (g + 1) * 512],
                            start=False, stop=(k == KH - 1))
                    banks.append(ps)
            return banks

        def gates_and_update(layer, banks, s_state):
            """r,z,n gate math; updates s_state in new buffer; returns
            (s_new_f32, sT_new_bf16)."""
            L = layer
            r = pgate.tile([B, H], BF16, tag=f"r{L}")
            z = pgate.tile([B, H], BF16, tag=f"z{L}")
            tt = pgate.tile([B, H], BF16, tag=f"tA{L}")
            u = pgate.tile([B, H], BF16, tag=f"tB{L}")
            n = pgate.tile([B, H], BF16, tag=f"n{L}")
            d = pgate.tile([B, H], BF16, tag=f"tA{L}", name=f"d{L}")
            zd = pgate.tile([B, H], BF16, tag=f"tB{L}", name=f"zd{L}")
            for g in range(2):
                nc.scalar.activation(r[:, bass.ts(g, 512)], banks[g][:], AF.Sigmoid)
            for g in range(2):
                nc.scalar.activation(z[:, bass.ts(g, 512)], banks[2 + g][:], AF.Sigmoid)
            for g in range(2):
                nc.vector.tensor_tensor(tt[:, bass.ts(g, 512)],
                                        r[:, bass.ts(g, 512)], banks[4 + g][:],
                                        ALU.mult)
            for g in range(2):
                nc.vector.tensor_tensor(u[:, bass.ts(g, 512)],
                                        tt[:, bass.ts(g, 512)], banks[6 + g][:],
                                        ALU.add)
            nc.scalar.activation(n[:], u[:], AF.Tanh)
            nc.vector.tensor_tensor(d[:], s_state[:], n[:], ALU.subtract)
            nc.vector.tensor_tensor(zd[:], z[:], d[:], ALU.mult)
            s_new = pstate.tile([B, H], BF16, tag=f"s{L}")
            nc.vector.tensor_tensor(s_new[:], n[:], zd[:], ALU.add)
            # transpose 8 chunks -> sT
            tp = ps_tp.tile([128, H], BF16, tag="tp")
            for k in range(KH):
                nc.tensor.transpose(tp[:, bass.ts(k, 128)],
                                    s_new[:, bass.ts(k, 128)], ident[:])
            sT = pstate.tile([128, H], BF16, tag=f"sT{L}")
            nc.vector.tensor_copy(sT[:], tp[:])
            return s_new, sT

        for t in range(n_steps):
            # one-hot of input token column t (layout [B, V] -> transpose -> [V, B])
            oh = psoft.tile([B, V], BF16, tag="oh")
            nc.vector.tensor_scalar(oh[:], iota64[:], x_sb[:, t:t + 1], None,
                                    ALU.is_equal)
            ohps = ps_oh.tile([V, 128], BF16, tag="ohps")
            nc.tensor.transpose(ohps[:], oh[:], ident[:])
            ohT = psoft.tile([V, 128], BF16, tag="ohT")
            nc.vector.tensor_copy(ohT[:], ohps[:])

            # ---- layer 0 ----
            banks0 = gate_banks(0, t, ohT, None, s0T_prev)
            s0, s0T = gates_and_update(0, banks0, s0)
            # ---- layer 1 ----
            banks1 = gate_banks(1, t, None, s0T, s1T_prev)
            s1, s1T = gates_and_update(1, banks1, s1)
            s0T_prev, s1T_prev = s0T, s1T

            # ---- FC + log_softmax + NLL ----
            fc = ps_fc.tile([128, V], F32, tag="fc")
            nc.tensor.matmul(fc[:, 0:V], ones_row[0:1, :], b_bfc[0:1, :],
                             start=True, stop=False)
            for k in range(KH):
                nc.tensor.matmul(fc[:, 0:V], s1T[:, bass.ts(k, 128)],
                                 w_fc[:, bass.ts(k, V)],
                                 start=False, stop=(k == KH - 1))
            m = psoft.tile([B, 1], F32, tag="m")
            nc.vector.reduce_max(m[:], fc[:, 0:V], axis=mybir.AxisListType.X)
            nm = psoft.tile([B, 1], F32, tag="nm")
            nc.vector.tensor_scalar_mul(nm[:], m[:], -1.0)
            ex = psoft.tile([B, V], F32, tag="ex")
            nc.scalar.activation(ex[:], fc[:, 0:V], AF.Exp, bias=nm[:, 0:1])
            sm = psoft.tile([B, 1], F32, tag="sm")
            nc.vector.reduce_sum(sm[:], ex[:], axis=mybir.AxisListType.X)
            ls = psoft.tile([B, 1], F32, tag="ls")
            nc.scalar.activation(ls[:], sm[:], AF.Ln)
            c = psoft.tile([B, 1], F32, tag="c")
            nc.vector.tensor_tensor(c[:], m[:], ls[:], ALU.add)
            lp = psoft.tile([B, V], F32, tag="lp")
            nc.vector.tensor_scalar(lp[:], fc[:, 0:V], c[:, 0:1], None, ALU.subtract)
            nc.sync.dma_start(d_lp[t], lp[:])
            # masked one-hot of target column t+1 (col0 masked via iota_m64)
            moh = psoft.tile([B, V], F32, tag="moh")
            nc.vector.tensor_scalar(moh[:], iota_m64[:], x_sb[:, t + 1:t + 2], None,
                                    ALU.is_equal)
            junk = psoft.tile([B, V], F32, tag="junk")
            nc.vector.tensor_tensor(junk[:], lp[:], moh[:], ALU.mult)
            tok = psoft.tile([B, 1], F32, tag="tok")
            nc.vector.reduce_sum(tok[:], junk[:], axis=mybir.AxisListType.X)
            acc_in, acc_out = nll[t % 2], nll[(t + 1) % 2]
            nc.vector.tensor_tensor(acc_out[:], acc_in[:], tok[:], ALU.subtract)

        # ---------- finale ----------
        nll_fin = nll[n_steps % 2]
        ne = pw.tile([B, S], F32, tag="ne")
        nc.vector.tensor_scalar(ne[:], x_sb[:], 0, None, ALU.not_equal)
        lens = pw.tile([B, 1], F32, tag="lens")
        nc.vector.reduce_sum(lens[:], ne[:], axis=mybir.AxisListType.X)
        inv = pw.tile([B, 1], F32, tag="inv")
        nc.vector.reciprocal(inv[:], lens[:])
        mol = pw.tile([B, 1], F32, tag="mol")
        nc.vector.tensor_tensor(mol[:], nll_fin[:], inv[:], ALU.mult)
        nc.sync.dma_start(d_mol[:], mol[:])
        lsps = ps_fc.tile([128, V], F32, tag="fc")
        nc.tensor.matmul(lsps[0:1, 0:1], nll_fin[:, 0:1], ones_col[:, 0:1],
                         start=True, stop=True)
        lsum = pw.tile([1, 1], F32, tag="lsum")
        nc.vector.tensor_copy(lsum[:], lsps[0:1, 0:1])
        nc.sync.dma_start(d_lsum[:], lsum[:])


# ---------------------------------------------------------------------------
# host side
# ---------------------------------------------------------------------------

def _chunked(w, kchunks):
    """[K, N] -> [128, kchunks*N] with chunk k at cols [k*N:(k+1)*N]."""
    K, Nn = w.shape
    assert K == kchunks * 128
    return np.ascontiguousarray(
        w.reshape(kchunks, 128, Nn).transpose(1, 0, 2).reshape(128, kchunks * Nn))


def _bf16(a):
    return np.asarray(a, np.float32).astype(ml_dtypes.bfloat16)


def prep_inputs(inputs):
    emb = np.asarray(inputs['emb'], np.float32)
    W_ih0 = np.asarray(inputs['W_ih0'], np.float32)
    W_hh0 = np.asarray(inputs['W_hh0'], np.float32)
    b_ih0 = np.asarray(inputs['b_ih0'], np.float32)
    b_hh0 = np.asarray(inputs['b_hh0'], np.float32)
    W_ih1 = np.asarray(inputs['W_ih1'], np.float32)
    W_hh1 = np.asarray(inputs['W_hh1'], np.float32)
    b_ih1 = np.asarray(inputs['b_ih1'], np.float32)
    b_hh1 = np.asarray(inputs['b_hh1'], np.float32)
    W_fc = np.asarray(inputs['W_fc'], np.float32)
    b_fc = np.asarray(inputs['b_fc'], np.float32)
    x = np.asarray(inputs['x'])

    brow0 = (b_ih0 + np.concatenate([b_hh0[:2 * H], np.zeros(H, np.float32)]))
    shared = {
        "embT": _bf16(_chunked(emb.T, E // 128)),
        "wih0": _bf16(_chunked(np.ascontiguousarray(W_ih0.T), E // 128)),
        "whh0": _bf16(_chunked(np.ascontiguousarray(W_hh0.T), KH)),
        "wih1": _bf16(_chunked(np.ascontiguousarray(W_ih1.T), KH)),
        "whh1": _bf16(_chunked(np.ascontiguousarray(W_hh1.T), KH)),
        "wfc": _bf16(_chunked(np.ascontiguousarray(W_fc.T), KH)),
        "brow0": _bf16(brow0)[None, :],
        "bhh0n": _bf16(b_hh0[2 * H:])[None, :],
        "brz1": _bf16((b_ih1 + b_hh1)[:2 * H])[None, :],
        "bn1": _bf16(b_ih1[2 * H:])[None, :],
        "bhh1n": _bf16(b_hh1[2 * H:])[None, :],
        "bfc": _bf16(b_fc)[None, :],
    }
    in_maps = []
    for c in range(NC):
        m = dict(shared)
        m["x"] = np.ascontiguousarray(x[c * B:(c + 1) * B].astype(np.float32))
        in_maps.append(m)
    return in_maps


def assemble_outputs(results):
    lp = np.concatenate(
        [r["lp"].transpose(1, 0, 2)[None] for r in results], axis=0
    ).reshape(N, T, V)
    mol = np.concatenate([r["mol"][:, 0] for r in results])
    loss = np.float32(sum(float(r["lsum"][0, 0]) for r in results) / N)
    return lp.astype(np.float32), mol.astype(np.float32), loss


_NC_CACHE = {}


def kernel(**inputs):
    key = "full"
    if key not in _NC_CACHE:
        _NC_CACHE[key] = build_kernel(T)
    nc = _NC_CACHE[key]
    in_maps = prep_inputs(inputs)
    res = run_bass_kernel_spmd(nc, in_maps, list(range(NC)))
    return assemble_outputs(res.results)


# revision 15
# speedup vs baseline: 1.0189x; 1.0189x over previous
"""Trainium2 Bass kernel for the 2-layer GRU language model.

Data-parallel over 8 NeuronCores: batch N=1024 sharded 128/core, weights
replicated. Per core, a fused per-step loop runs both GRU layers + the
FC/log-softmax/NLL tail; layer-0's input projection is pre-folded into a
64x3072 table (M0b = emb @ W_ih0.T + biases) gathered via one-hot matmuls.
All big GEMMs in bf16 (fp32 accumulate), gates/softmax in fp32/bf16 mix.
"""
import numpy as np
import ml_dtypes

import concourse.bass as bass
import concourse.mybir as mybir
import concourse.tile as tile
from concourse.bass_utils import run_bass_kernel_spmd

BF16 = mybir.dt.bfloat16
F32 = mybir.dt.float32
I32 = mybir.dt.int32
AF = mybir.ActivationFunctionType
ALU = mybir.AluOpType

N, S = 1024, 128
V, E, H = 64, 512, 1024
B = 128           # per-core batch
T = S - 1         # 127 steps
NC = 8
KH = H // 128     # 8 K-chunks of H
G3 = 3 * H        # 3072


def split_drain_waits(nc):
    """walrus here accepts only 1 sync wait per instruction; hoist extras
    onto preceding NoOps on the same engine."""
    for f in nc.m.functions:
        for blk in f.blocks:
            new_list, changed = [], False
            for inst in blk.instructions:
                si = getattr(inst, "sync_info", None)
                if (si is not None and len(si.on_wait) > 1):
                    for k, w in enumerate(si.on_wait[:-1]):
                        nop = mybir.InstNoOp(
                            name=f"{inst.name}-w{k}",
                            sync_info=mybir.SyncInfo(on_wait=[w], on_update=[]),
                            bass_nofuse=True,
                            engine=inst.engine,
                        )
                        new_list.append(nop)
                    inst.sync_info = mybir.SyncInfo(
                        on_wait=[si.on_wait[-1]], on_update=si.on_update)
                    changed = True
                new_list.append(inst)
            if changed:
                blk.instructions = new_list


def build_kernel(n_steps=T, split_drains=True, repeat=None):
    nc = bass.Bass()
    dp = nc.declare_dram_parameter
    d_x = dp("x", [B, S], F32, isOutput=False)
    d_embT = dp("embT", [128, (E // 128) * V], BF16, isOutput=False)
    d_wih0 = dp("wih0", [128, (E // 128) * G3], BF16, isOutput=False)
    d_whh0 = dp("whh0", [128, KH * G3], BF16, isOutput=False)
    d_wih1 = dp("wih1", [128, KH * G3], BF16, isOutput=False)
    d_whh1 = dp("whh1", [128, KH * G3], BF16, isOutput=False)
    d_wfc = dp("wfc", [128, KH * V], BF16, isOutput=False)
    d_brow0 = dp("brow0", [1, G3], BF16, isOutput=False)   # bih0 + bhh0(rz only)
    d_bhh0n = dp("bhh0n", [1, H], BF16, isOutput=False)
    d_brz1 = dp("brz1", [1, 2 * H], BF16, isOutput=False)  # bih1+bhh1 rz
    d_bn1 = dp("bn1", [1, H], BF16, isOutput=False)        # bih1 n-part
    d_bhh1n = dp("bhh1n", [1, H], BF16, isOutput=False)
    d_bfc = dp("bfc", [1, V], BF16, isOutput=False)
    d_lp = dp("lp", [T, B, V], F32, isOutput=True)
    d_mol = dp("mol", [B, 1], F32, isOutput=True)
    d_lsum = dp("lsum", [1, 1], F32, isOutput=True)

    with tile.TileContext(nc) as tc:
        _body(nc, tc, n_steps,
              d_x, d_embT, d_wih0, d_whh0, d_wih1, d_whh1, d_wfc,
              d_brow0, d_bhh0n, d_brz1, d_bn1, d_bhh1n, d_bfc,
              d_lp, d_mol, d_lsum, repeat=repeat)
    if split_drains:
        split_drain_waits(nc)
    return nc


def _body(nc, tc, n_steps,
          d_x, d_embT, d_wih0, d_whh0, d_wih1, d_whh1, d_wfc,
          d_brow0, d_bhh0n, d_brz1, d_bn1, d_bhh1n, d_bfc,
          d_lp, d_mol, d_lsum, repeat=None):
    from contextlib import ExitStack
    ctx = ExitStack()
    with ctx:
        # ---------- persistent pool (init pools created inline, then the
        # steady-state pools, so init scratch space is reused) ----------
        pw = ctx.enter_context(tc.tile_pool(name="pw", bufs=1))       # persistent
        if repeat is not None:
            ctx.enter_context(tc.For_i(0, repeat, 1, name="rep"))

        # ---------- persistent loads ----------
        x_sb = pw.tile([B, S], F32, tag="x")
        nc.sync.dma_start(x_sb[:], d_x[:])
        w_hh0 = pw.tile([128, KH * G3], BF16, tag="whh0")
        w_ih1 = pw.tile([128, KH * G3], BF16, tag="wih1")
        w_hh1 = pw.tile([128, KH * G3], BF16, tag="whh1")
        for k in range(KH):
            nc.sync.dma_start(w_hh0[:, bass.ts(k, G3)], d_whh0[:, bass.ts(k, G3)])
        for k in range(KH):
            nc.sync.dma_start(w_ih1[:, bass.ts(k, G3)], d_wih1[:, bass.ts(k, G3)])
        for k in range(KH):
            nc.sync.dma_start(w_hh1[:, bass.ts(k, G3)], d_whh1[:, bass.ts(k, G3)])
        w_fc = pw.tile([128, KH * V], BF16, tag="wfc")
        nc.sync.dma_start(w_fc[:], d_wfc[:])
        b_bhh0n = pw.tile([1, H], BF16, tag="bhh0n")
        nc.sync.dma_start(b_bhh0n[:], d_bhh0n[:])
        b_brz1 = pw.tile([1, 2 * H], BF16, tag="brz1")
        nc.sync.dma_start(b_brz1[:], d_brz1[:])
        b_bn1 = pw.tile([1, H], BF16, tag="bn1")
        nc.sync.dma_start(b_bn1[:], d_bn1[:])
        b_bhh1n = pw.tile([1, H], BF16, tag="bhh1n")
        nc.sync.dma_start(b_bhh1n[:], d_bhh1n[:])
        b_bfc = pw.tile([1, V], BF16, tag="bfc")
        nc.sync.dma_start(b_bfc[:], d_bfc[:])

        # ---------- constants ----------
        iota64 = pw.tile([B, V], F32, tag="iota64")
        nc.gpsimd.iota(iota64[:], pattern=[[1, V]], base=0, channel_multiplier=0, allow_small_or_imprecise_dtypes=True)
        iota_m64 = pw.tile([B, V], F32, tag="iotam64")
        nc.gpsimd.iota(iota_m64[:], pattern=[[1, V]], base=0, channel_multiplier=0, allow_small_or_imprecise_dtypes=True)
        nc.gpsimd.memset(iota_m64[:, 0:1], -1.0)  # col0 never matches -> masks tgt==0
        iota128 = pw.tile([128, 128], F32, tag="iota128")
        nc.gpsimd.iota(iota128[:], pattern=[[1, 128]], base=0, channel_multiplier=0, allow_small_or_imprecise_dtypes=True)
        iota_col = pw.tile([128, 1], F32, tag="iotacol")
        nc.gpsimd.iota(iota_col[:], pattern=[[0, 1]], base=0, channel_multiplier=1, allow_small_or_imprecise_dtypes=True)
        ident = pw.tile([128, 128], BF16, tag="ident")
        nc.vector.tensor_scalar(ident[:], iota128[:], iota_col[:, 0:1], None,
                                ALU.is_equal)
        ones_row = pw.tile([1, 128], BF16, tag="onesrow")
        nc.vector.memset(ones_row[:], 1.0)
        ones_col = pw.tile([128, 1], F32, tag="onescol")
        nc.vector.memset(ones_col[:], 1.0)

        # ---------- M0b table: emb @ W_ih0.T + brow0 ----------
        m0b = pw.tile([V, G3], BF16, tag="m0b")
        with tc.tile_pool(name="pinit", bufs=2) as pinit, \
             tc.tile_pool(name="ps_m0", bufs=6, space="PSUM") as ps_m0:
            embT = pinit.tile([128, (E // 128) * V], BF16, tag="embT", bufs=1)
            nc.sync.dma_start(embT[:], d_embT[:])
            b_brow0 = pinit.tile([1, G3], BF16, tag="brow0", bufs=1)
            nc.sync.dma_start(b_brow0[:], d_brow0[:])
            m0ps = [ps_m0.tile([128, 512], F32, tag="m0", name=f"m0ps{g}")
                    for g in range(G3 // 512)]
            for k in range(E // 128):
                wih0_k = pinit.tile([128, G3], BF16, tag="wih0k")
                nc.sync.dma_start(wih0_k[:], d_wih0[:, bass.ts(k, G3)])
                for g in range(G3 // 512):
                    ps = m0ps[g]
                    if k == 0:
                        nc.tensor.matmul(ps[0:V, :], ones_row[0:1, 0:V],
                                         b_brow0[0:1, bass.ts(g, 512)],
                                         start=True, stop=False)
                    nc.tensor.matmul(ps[0:V, :], embT[:, bass.ts(k, V)],
                                     wih0_k[:, bass.ts(g, 512)],
                                     start=False, stop=(k == E // 128 - 1))
            for g in range(G3 // 512):
                nc.vector.tensor_copy(m0b[:, bass.ts(g, 512)], m0ps[g][0:V, :])

        # ---------- steady-state pools ----------
        pstate = ctx.enter_context(tc.tile_pool(name="pstate", bufs=2))
        pgate = ctx.enter_context(tc.tile_pool(name="pgate", bufs=1))
        psoft = ctx.enter_context(tc.tile_pool(name="psoft", bufs=2))
        ps_gb = ctx.enter_context(tc.tile_pool(name="ps_gb", bufs=6, space="PSUM"))
        ps_tp = ctx.enter_context(tc.tile_pool(name="ps_tp", bufs=1, space="PSUM"))
        ps_sm = ctx.enter_context(tc.tile_pool(name="ps_sm", bufs=1, space="PSUM"))
        ps_fc = ps_sm
        ps_oh = ps_sm

        # ---------- states / accumulators ----------
        s0 = pw.tile([B, H], BF16, tag="s0i")
        nc.vector.memset(s0[:], 0.0)
        s1 = pw.tile([B, H], BF16, tag="s1i")
        nc.vector.memset(s1[:], 0.0)
        nll = [pw.tile([B, 1], F32, tag=f"nll{i}", name=f"nll{i}") for i in range(2)]
        nc.vector.memset(nll[0][:], 0.0)
        s0T_prev = None
        s1T_prev = None

        def gate_banks(layer, t, ohT, s_inT, s_recT):
            """Emit the 8 psum gate banks for one layer, k-outer (stationary
            reuse). Returns banks in order [r0,r1,z0,z1,hn0,hn1,xn0,xn1]."""
            if layer == 0:
                rz = [ps_gb.tile([128, 512], F32, tag="gb", name=f"l0rz{g}")
                      for g in range(4)]
                hn = [ps_gb.tile([128, 512], F32, tag="gb", name=f"l0hn{g}")
                      for g in range(2)]
                # bias first on hn banks (b_hh0n sits inside r*(...))
                for g in range(2):
                    nc.tensor.matmul(hn[g][:], ones_row[0:1, :],
                                     b_bhh0n[0:1, bass.ts(g, 512)],
                                     start=True, stop=(t == 0))
                if t > 0:
                    for k in range(KH):
                        for gi, ps in enumerate(rz + hn):
                            col = k * G3 + gi * 512
                            nc.tensor.matmul(
                                ps[:], s_recT[:, bass.ts(k, 128)],
                                w_hh0[:, col:col + 512],
                                start=(k == 0 and gi < 4),
                                stop=(k == KH - 1 and gi >= 4))
                # one-hot gather last (ohT may arrive late; rz accumulation
                # finishes with it)
                for g in range(4):
                    nc.tensor.matmul(rz[g][:], ohT[:], m0b[:, bass.ts(g, 512)],
                                     start=(t == 0), stop=True)
                xn = []
                for g in range(2):
                    ps = ps_gb.tile([128, 512], F32, tag="gb", name=f"l0xn{g}")
                    nc.tensor.matmul(ps[:], ohT[:],
                                     m0b[:, 2048 + g * 512:2048 + (g + 1) * 512],
                                     start=True, stop=True)
                    xn.append(ps)
                return rz + hn + xn
            else:
                banks = {}
                # two groups of [rz, rz, hn, xn] to bound live psum at 4
                for half in range(2):
                    rzA = [ps_gb.tile([128, 512], F32, tag="gb",
                                      name=f"l1rz{half}{j}") for j in range(2)]
                    hnA = ps_gb.tile([128, 512], F32, tag="gb", name=f"l1hn{half}")
                    xnA = ps_gb.tile([128, 512], F32, tag="gb", name=f"l1xn{half}")
                    g0 = 2 * half           # rz bank indices g0, g0+1
                    # biases open every accumulation
                    for j in range(2):
                        nc.tensor.matmul(rzA[j][:], ones_row[0:1, :],
                                         b_brz1[0:1, bass.ts(g0 + j, 512)],
                                         start=True, stop=False)
                    nc.tensor.matmul(hnA[:], ones_row[0:1, :],
                                     b_bhh1n[0:1, bass.ts(half, 512)],
                                     start=True, stop=(t == 0))
                    nc.tensor.matmul(xnA[:], ones_row[0:1, :],
                                     b_bn1[0:1, bass.ts(half, 512)],
                                     start=True, stop=False)
                    # recurrent part (gh1): rz pair + hn
                    if t > 0:
                        for k in range(KH):
                            for j in range(2):
                                col = k * G3 + (g0 + j) * 512
                                nc.tensor.matmul(rzA[j][:],
                                                 s_recT[:, bass.ts(k, 128)],
                                                 w_hh1[:, col:col + 512],
                                                 start=False, stop=False)
                            col = k * G3 + 2048 + half * 512
                            nc.tensor.matmul(hnA[:], s_recT[:, bass.ts(k, 128)],
                                             w_hh1[:, col:col + 512],
                                             start=False, stop=(k == KH - 1))
                    # input part (gx1): rz pair + xn
                    for k in range(KH):
                        for j in range(2):
                            col = k * G3 + (g0 + j) * 512
                            nc.tensor.matmul(rzA[j][:], s_inT[:, bass.ts(k, 128)],
                                             w_ih1[:, col:col + 512],
                                             start=False, stop=(k == KH - 1))
                        col = k * G3 + 2048 + half * 512
                        nc.tensor.matmul(xnA[:], s_inT[:, bass.ts(k, 128)],
                                         w_ih1[:, col:col + 512],
                                         start=False, stop=(k == KH - 1))
                    banks[f"rz{g0}"] = rzA[0]
                    banks[f"rz{g0+1}"] = rzA[1]
                    banks[f"hn{half}"] = hnA
                    banks[f"xn{half}"] = xnA
                return [banks["rz0"], banks["rz1"], banks["rz2"], banks["rz3"],
                        banks["hn0"], banks["hn1"], banks["xn0"], banks["xn1"]]

        def gates_and_update(layer, banks, s_state):
            """r,z,n gate math; updates s_state in new buffer; returns
            (s_new_f32, sT_new_bf16)."""
            L = layer
            r = pgate.tile([B, H], BF16, tag=f"r{L}")
            z = pgate.tile([B, H], BF16, tag=f"z{L}")
            tt = pgate.tile([B, H], BF16, tag=f"tA{L}")
            u = pgate.tile([B, H], BF16, tag=f"tB{L}")
            n = pgate.tile([B, H], BF16, tag=f"n{L}")
            d = pgate.tile([B, H], BF16, tag=f"tA{L}", name=f"d{L}")
            zd = pgate.tile([B, H], BF16, tag=f"tB{L}", name=f"zd{L}")
            for g in range(2):
                nc.scalar.activation(r[:, bass.ts(g, 512)], banks[g][:], AF.Sigmoid)
            for g in range(2):
                nc.scalar.activation(z[:, bass.ts(g, 512)], banks[2 + g][:], AF.Sigmoid)
            for g in range(2):
                nc.vector.tensor_tensor(tt[:, bass.ts(g, 512)],
                                        r[:, bass.ts(g, 512)], banks[4 + g][:],
                                        ALU.mult)
            for g in range(2):
                nc.vector.tensor_tensor(u[:, bass.ts(g, 512)],
                                        tt[:, bass.ts(g, 512)], banks[6 + g][:],
                                        ALU.add)
            nc.scalar.activation(n[:], u[:], AF.Tanh)
            nc.vector.tensor_tensor(d[:], s_state[:], n[:], ALU.subtract)
            nc.vector.tensor_tensor(zd[:], z[:], d[:], ALU.mult)
            s_new = pstate.tile([B, H], BF16, tag=f"s{L}")
            nc.vector.tensor_tensor(s_new[:], n[:], zd[:], ALU.add)
            # transpose 8 chunks -> sT
            tp = ps_tp.tile([128, H], BF16, tag="tp")
            for k in range(KH):
                nc.tensor.transpose(tp[:, bass.ts(k, 128)],
                                    s_new[:, bass.ts(k, 128)], ident[:])
            sT = pstate.tile([128, H], BF16, tag=f"sT{L}")
            nc.vector.tensor_copy(sT[:], tp[:])
            return s_new, sT

        for t in range(n_steps):
            # one-hot of input token column t (layout [B, V] -> transpose -> [V, B])
            oh = psoft.tile([B, V], BF16, tag="oh")
            nc.vector.tensor_scalar(oh[:], iota64[:], x_sb[:, t:t + 1], None,
                                    ALU.is_equal)
            ohps = ps_oh.tile([V, 128], BF16, tag="sm")
            nc.tensor.transpose(ohps[:], oh[:], ident[:])
            ohT = psoft.tile([V, 128], BF16, tag="ohT")
            nc.vector.tensor_copy(ohT[:], ohps[:])

            # ---- layer 0 ----
            banks0 = gate_banks(0, t, ohT, None, s0T_prev)
            s0, s0T = gates_and_update(0, banks0, s0)
            # ---- layer 1 ----
            banks1 = gate_banks(1, t, None, s0T, s1T_prev)
            s1, s1T = gates_and_update(1, banks1, s1)
            s0T_prev, s1T_prev = s0T, s1T

            # ---- FC + log_softmax + NLL ----
            fc = ps_fc.tile([128, V], F32, tag="sm", name="fc")
            nc.tensor.matmul(fc[:, 0:V], ones_row[0:1, :], b_bfc[0:1, :],
                             start=True, stop=False)
            for k in range(KH):
                nc.tensor.matmul(fc[:, 0:V], s1T[:, bass.ts(k, 128)],
                                 w_fc[:, bass.ts(k, V)],
                                 start=False, stop=(k == KH - 1))
            m = psoft.tile([B, 1], F32, tag="m")
            nc.vector.reduce_max(m[:], fc[:, 0:V], axis=mybir.AxisListType.X)
            nm = psoft.tile([B, 1], F32, tag="nm")
            nc.vector.tensor_scalar_mul(nm[:], m[:], -1.0)
            ex = psoft.tile([B, V], F32, tag="ex")
            nc.scalar.activation(ex[:], fc[:, 0:V], AF.Exp, bias=nm[:, 0:1])
            sm = psoft.tile([B, 1], F32, tag="sm")
            nc.vector.reduce_sum(sm[:], ex[:], axis=mybir.AxisListType.X)
            ls = psoft.tile([B, 1], F32, tag="ls")
            nc.scalar.activation(ls[:], sm[:], AF.Ln)
            c = psoft.tile([B, 1], F32, tag="c")
            nc.vector.tensor_tensor(c[:], m[:], ls[:], ALU.add)
            lp = psoft.tile([B, V], F32, tag="lp")
            nc.vector.tensor_scalar(lp[:], fc[:, 0:V], c[:, 0:1], None, ALU.subtract)
            nc.sync.dma_start(d_lp[t], lp[:])
            # masked one-hot of target column t+1 (col0 masked via iota_m64)
            moh = psoft.tile([B, V], F32, tag="moh")
            nc.vector.tensor_scalar(moh[:], iota_m64[:], x_sb[:, t + 1:t + 2], None,
                                    ALU.is_equal)
            junk = psoft.tile([B, V], F32, tag="junk")
            nc.vector.tensor_tensor(junk[:], lp[:], moh[:], ALU.mult)
            tok = psoft.tile([B, 1], F32, tag="tok")
            nc.vector.reduce_sum(tok[:], junk[:], axis=mybir.AxisListType.X)
            acc_in, acc_out = nll[t % 2], nll[(t + 1) % 2]
            nc.vector.tensor_tensor(acc_out[:], acc_in[:], tok[:], ALU.subtract)

        # ---------- finale ----------
        nll_fin = nll[n_steps % 2]
        ne = pw.tile([B, S], F32, tag="ne")
        nc.vector.tensor_scalar(ne[:], x_sb[:], 0, None, ALU.not_equal)
        lens = pw.tile([B, 1], F32, tag="lens")
        nc.vector.reduce_sum(lens[:], ne[:], axis=mybir.AxisListType.X)
        inv = pw.tile([B, 1], F32, tag="inv")
        nc.vector.reciprocal(inv[:], lens[:])
        mol = pw.tile([B, 1], F32, tag="mol")
        nc.vector.tensor_tensor(mol[:], nll_fin[:], inv[:], ALU.mult)
        nc.sync.dma_start(d_mol[:], mol[:])
        lsps = ps_fc.tile([128, V], F32, tag="sm", name="lsps")
        nc.tensor.matmul(lsps[0:1, 0:1], nll_fin[:, 0:1], ones_col[:, 0:1],
                         start=True, stop=True)
        lsum = pw.tile([1, 1], F32, tag="lsum")
        nc.vector.tensor_copy(lsum[:], lsps[0:1, 0:1])
        nc.sync.dma_start(d_lsum[:], lsum[:])


# ---------------------------------------------------------------------------
# host side
# ---------------------------------------------------------------------------

def _chunked(w, kchunks):
    """[K, N] -> [128, kchunks*N] with chunk k at cols [k*N:(k+1)*N]."""
    K, Nn = w.shape
    assert K == kchunks * 128
    return np.ascontiguousarray(
        w.reshape(kchunks, 128, Nn).transpose(1, 0, 2).reshape(128, kchunks * Nn))


def _bf16(a):
    return np.asarray(a, np.float32).astype(ml_dtypes.bfloat16)


def prep_inputs(inputs):
    emb = np.asarray(inputs['emb'], np.float32)
    W_ih0 = np.asarray(inputs['W_ih0'], np.float32)
    W_hh0 = np.asarray(inputs['W_hh0'], np.float32)
    b_ih0 = np.asarray(inputs['b_ih0'], np.float32)
    b_hh0 = np.asarray(inputs['b_hh0'], np.float32)
    W_ih1 = np.asarray(inputs['W_ih1'], np.float32)
    W_hh1 = np.asarray(inputs['W_hh1'], np.float32)
    b_ih1 = np.asarray(inputs['b_ih1'], np.float32)
    b_hh1 = np.asarray(inputs['b_hh1'], np.float32)
    W_fc = np.asarray(inputs['W_fc'], np.float32)
    b_fc = np.asarray(inputs['b_fc'], np.float32)
    x = np.asarray(inputs['x'])

    brow0 = (b_ih0 + np.concatenate([b_hh0[:2 * H], np.zeros(H, np.float32)]))
    shared = {
        "embT": _bf16(_chunked(emb.T, E // 128)),
        "wih0": _bf16(_chunked(np.ascontiguousarray(W_ih0.T), E // 128)),
        "whh0": _bf16(_chunked(np.ascontiguousarray(W_hh0.T), KH)),
        "wih1": _bf16(_chunked(np.ascontiguousarray(W_ih1.T), KH)),
        "whh1": _bf16(_chunked(np.ascontiguousarray(W_hh1.T), KH)),
        "wfc": _bf16(_chunked(np.ascontiguousarray(W_fc.T), KH)),
        "brow0": _bf16(brow0)[None, :],
        "bhh0n": _bf16(b_hh0[2 * H:])[None, :],
        "brz1": _bf16((b_ih1 + b_hh1)[:2 * H])[None, :],
        "bn1": _bf16(b_ih1[2 * H:])[None, :],
        "bhh1n": _bf16(b_hh1[2 * H:])[None, :],
        "bfc": _bf16(b_fc)[None, :],
    }
    in_maps = []
    for c in range(NC):
        m = dict(shared)
        m["x"] = np.ascontiguousarray(x[c * B:(c + 1) * B].astype(np.float32))
        in_maps.append(m)
    return in_maps


def assemble_outputs(results):
    lp = np.concatenate(
        [r["lp"].transpose(1, 0, 2)[None] for r in results], axis=0
    ).reshape(N, T, V)
    mol = np.concatenate([r["mol"][:, 0] for r in results])
    loss = np.float32(sum(float(r["lsum"][0, 0]) for r in results) / N)
    return lp.astype(np.float32), mol.astype(np.float32), loss


_NC_CACHE = {}


def kernel(**inputs):
    key = "full"
    if key not in _NC_CACHE:
        _NC_CACHE[key] = build_kernel(T)
    nc = _NC_CACHE[key]
    in_maps = prep_inputs(inputs)
    res = run_bass_kernel_spmd(nc, in_maps, list(range(NC)))
    return assemble_outputs(res.results)


# revision 16
# speedup vs baseline: 1.0236x; 1.0046x over previous
"""Trainium2 Bass kernel for the 2-layer GRU language model.

Data-parallel over 8 NeuronCores: batch N=1024 sharded 128/core, weights
replicated. Per core, a fused per-step loop runs both GRU layers + the
FC/log-softmax/NLL tail; layer-0's input projection is pre-folded into a
64x3072 table (M0b = emb @ W_ih0.T + biases) gathered via one-hot matmuls.
All big GEMMs in bf16 (fp32 accumulate), gates/softmax in fp32/bf16 mix.
"""
import numpy as np
import ml_dtypes

import concourse.bass as bass
import concourse.mybir as mybir
import concourse.tile as tile
from concourse.bass_utils import run_bass_kernel_spmd

BF16 = mybir.dt.bfloat16
F32 = mybir.dt.float32
I32 = mybir.dt.int32
AF = mybir.ActivationFunctionType
ALU = mybir.AluOpType

N, S = 1024, 128
V, E, H = 64, 512, 1024
B = 128           # per-core batch
T = S - 1         # 127 steps
NC = 8
KH = H // 128     # 8 K-chunks of H
G3 = 3 * H        # 3072


def split_drain_waits(nc):
    """walrus here accepts only 1 sync wait per instruction; hoist extras
    onto preceding NoOps on the same engine."""
    for f in nc.m.functions:
        for blk in f.blocks:
            new_list, changed = [], False
            for inst in blk.instructions:
                si = getattr(inst, "sync_info", None)
                if (si is not None and len(si.on_wait) > 1):
                    for k, w in enumerate(si.on_wait[:-1]):
                        nop = mybir.InstNoOp(
                            name=f"{inst.name}-w{k}",
                            sync_info=mybir.SyncInfo(on_wait=[w], on_update=[]),
                            bass_nofuse=True,
                            engine=inst.engine,
                        )
                        new_list.append(nop)
                    inst.sync_info = mybir.SyncInfo(
                        on_wait=[si.on_wait[-1]], on_update=si.on_update)
                    changed = True
                new_list.append(inst)
            if changed:
                blk.instructions = new_list


def build_kernel(n_steps=T, split_drains=True, repeat=None):
    nc = bass.Bass()
    dp = nc.declare_dram_parameter
    d_x = dp("x", [B, S], F32, isOutput=False)
    d_embT = dp("embT", [128, (E // 128) * V], BF16, isOutput=False)
    d_wih0 = dp("wih0", [128, (E // 128) * G3], BF16, isOutput=False)
    d_whh0 = dp("whh0", [128, KH * G3], BF16, isOutput=False)
    d_wih1 = dp("wih1", [128, KH * G3], BF16, isOutput=False)
    d_whh1 = dp("whh1", [128, KH * G3], BF16, isOutput=False)
    d_wfc = dp("wfc", [128, KH * V], BF16, isOutput=False)
    d_brow0 = dp("brow0", [1, G3], BF16, isOutput=False)   # bih0 + bhh0(rz only)
    d_bhh0n = dp("bhh0n", [1, H], BF16, isOutput=False)
    d_brz1 = dp("brz1", [1, 2 * H], BF16, isOutput=False)  # bih1+bhh1 rz
    d_bn1 = dp("bn1", [1, H], BF16, isOutput=False)        # bih1 n-part
    d_bhh1n = dp("bhh1n", [1, H], BF16, isOutput=False)
    d_bfc = dp("bfc", [1, V], BF16, isOutput=False)
    d_lp = dp("lp", [T, B, V], F32, isOutput=True)
    d_mol = dp("mol", [B, 1], F32, isOutput=True)
    d_lsum = dp("lsum", [1, 1], F32, isOutput=True)

    with tile.TileContext(nc) as tc:
        _body(nc, tc, n_steps,
              d_x, d_embT, d_wih0, d_whh0, d_wih1, d_whh1, d_wfc,
              d_brow0, d_bhh0n, d_brz1, d_bn1, d_bhh1n, d_bfc,
              d_lp, d_mol, d_lsum, repeat=repeat)
    if split_drains:
        split_drain_waits(nc)
    return nc


def _body(nc, tc, n_steps,
          d_x, d_embT, d_wih0, d_whh0, d_wih1, d_whh1, d_wfc,
          d_brow0, d_bhh0n, d_brz1, d_bn1, d_bhh1n, d_bfc,
          d_lp, d_mol, d_lsum, repeat=None):
    from contextlib import ExitStack
    ctx = ExitStack()
    with ctx:
        # ---------- persistent pool (init pools created inline, then the
        # steady-state pools, so init scratch space is reused) ----------
        pw = ctx.enter_context(tc.tile_pool(name="pw", bufs=1))       # persistent
        if repeat is not None:
            ctx.enter_context(tc.For_i(0, repeat, 1, name="rep"))

        # ---------- persistent loads ----------
        x_sb = pw.tile([B, S], F32, tag="x")
        nc.sync.dma_start(x_sb[:], d_x[:])
        w_hh0 = pw.tile([128, KH * G3], BF16, tag="whh0")
        w_ih1 = pw.tile([128, KH * G3], BF16, tag="wih1")
        w_hh1 = pw.tile([128, KH * G3], BF16, tag="whh1")
        for k in range(KH):
            nc.sync.dma_start(w_hh0[:, bass.ts(k, G3)], d_whh0[:, bass.ts(k, G3)])
        for k in range(KH):
            nc.sync.dma_start(w_ih1[:, bass.ts(k, G3)], d_wih1[:, bass.ts(k, G3)])
        for k in range(KH):
            nc.sync.dma_start(w_hh1[:, bass.ts(k, G3)], d_whh1[:, bass.ts(k, G3)])
        w_fc = pw.tile([128, KH * V], BF16, tag="wfc")
        nc.sync.dma_start(w_fc[:], d_wfc[:])
        b_bhh0n = pw.tile([1, H], BF16, tag="bhh0n")
        nc.sync.dma_start(b_bhh0n[:], d_bhh0n[:])
        b_brz1 = pw.tile([1, 2 * H], BF16, tag="brz1")
        nc.sync.dma_start(b_brz1[:], d_brz1[:])
        b_bn1 = pw.tile([1, H], BF16, tag="bn1")
        nc.sync.dma_start(b_bn1[:], d_bn1[:])
        b_bhh1n = pw.tile([1, H], BF16, tag="bhh1n")
        nc.sync.dma_start(b_bhh1n[:], d_bhh1n[:])
        b_bfc = pw.tile([1, V], BF16, tag="bfc")
        nc.sync.dma_start(b_bfc[:], d_bfc[:])

        # ---------- constants ----------
        iota64 = pw.tile([B, V], F32, tag="iota64")
        nc.gpsimd.iota(iota64[:], pattern=[[1, V]], base=0, channel_multiplier=0, allow_small_or_imprecise_dtypes=True)
        iota_m64 = pw.tile([B, V], F32, tag="iotam64")
        nc.gpsimd.iota(iota_m64[:], pattern=[[1, V]], base=0, channel_multiplier=0, allow_small_or_imprecise_dtypes=True)
        nc.gpsimd.memset(iota_m64[:, 0:1], -1.0)  # col0 never matches -> masks tgt==0
        iota128 = pw.tile([128, 128], F32, tag="iota128")
        nc.gpsimd.iota(iota128[:], pattern=[[1, 128]], base=0, channel_multiplier=0, allow_small_or_imprecise_dtypes=True)
        iota_col = pw.tile([128, 1], F32, tag="iotacol")
        nc.gpsimd.iota(iota_col[:], pattern=[[0, 1]], base=0, channel_multiplier=1, allow_small_or_imprecise_dtypes=True)
        ident = pw.tile([128, 128], BF16, tag="ident")
        nc.vector.tensor_scalar(ident[:], iota128[:], iota_col[:, 0:1], None,
                                ALU.is_equal)
        ones_row = pw.tile([1, 128], BF16, tag="onesrow")
        nc.vector.memset(ones_row[:], 1.0)
        ones_col = pw.tile([128, 1], F32, tag="onescol")
        nc.vector.memset(ones_col[:], 1.0)

        # ---------- M0b table: emb @ W_ih0.T + brow0 ----------
        m0b = pw.tile([V, G3], BF16, tag="m0b")
        with tc.tile_pool(name="pinit", bufs=2) as pinit, \
             tc.tile_pool(name="ps_m0", bufs=6, space="PSUM") as ps_m0:
            embT = pinit.tile([128, (E // 128) * V], BF16, tag="embT", bufs=1)
            nc.sync.dma_start(embT[:], d_embT[:])
            b_brow0 = pinit.tile([1, G3], BF16, tag="brow0", bufs=1)
            nc.sync.dma_start(b_brow0[:], d_brow0[:])
            m0ps = [ps_m0.tile([128, 512], F32, tag="m0", name=f"m0ps{g}")
                    for g in range(G3 // 512)]
            for k in range(E // 128):
                wih0_k = pinit.tile([128, G3], BF16, tag="wih0k")
                nc.sync.dma_start(wih0_k[:], d_wih0[:, bass.ts(k, G3)])
                for g in range(G3 // 512):
                    ps = m0ps[g]
                    if k == 0:
                        nc.tensor.matmul(ps[0:V, :], ones_row[0:1, 0:V],
                                         b_brow0[0:1, bass.ts(g, 512)],
                                         start=True, stop=False)
                    nc.tensor.matmul(ps[0:V, :], embT[:, bass.ts(k, V)],
                                     wih0_k[:, bass.ts(g, 512)],
                                     start=False, stop=(k == E // 128 - 1))
            for g in range(G3 // 512):
                nc.vector.tensor_copy(m0b[:, bass.ts(g, 512)], m0ps[g][0:V, :])

        # ---------- steady-state pools ----------
        pstate = ctx.enter_context(tc.tile_pool(name="pstate", bufs=2))
        pgate = ctx.enter_context(tc.tile_pool(name="pgate", bufs=1))
        psoft = ctx.enter_context(tc.tile_pool(name="psoft", bufs=2))
        ps_gb = ctx.enter_context(tc.tile_pool(name="ps_gb", bufs=5, space="PSUM"))
        ps_tp = ctx.enter_context(tc.tile_pool(name="ps_tp", bufs=1, space="PSUM"))
        ps_fc = ctx.enter_context(tc.tile_pool(name="ps_fc", bufs=1, space="PSUM"))
        ps_oh = ctx.enter_context(tc.tile_pool(name="ps_oh", bufs=1, space="PSUM"))

        # ---------- states / accumulators ----------
        s0 = pw.tile([B, H], BF16, tag="s0i")
        nc.vector.memset(s0[:], 0.0)
        s1 = pw.tile([B, H], BF16, tag="s1i")
        nc.vector.memset(s1[:], 0.0)
        nll = [pw.tile([B, 1], F32, tag=f"nll{i}", name=f"nll{i}") for i in range(2)]
        nc.vector.memset(nll[0][:], 0.0)
        s0T_prev = None
        s1T_prev = None

        def gate_banks(layer, t, ohT, s_inT, s_recT):
            """Emit the 8 psum gate banks for one layer, k-outer (stationary
            reuse). Returns banks in order [r0,r1,z0,z1,hn0,hn1,xn0,xn1]."""
            if layer == 0:
                rz = [None] * 4
                hn = [None] * 2
                for half in range(2):
                    rzA = [ps_gb.tile([128, 512], F32, tag="gb",
                                      name=f"l0rz{half}{j}") for j in range(2)]
                    hnA = ps_gb.tile([128, 512], F32, tag="gb", name=f"l0hn{half}")
                    g0 = 2 * half
                    nc.tensor.matmul(hnA[:], ones_row[0:1, :],
                                     b_bhh0n[0:1, bass.ts(half, 512)],
                                     start=True, stop=(t == 0))
                    if t > 0:
                        for k in range(KH):
                            for j in range(2):
                                col = k * G3 + (g0 + j) * 512
                                nc.tensor.matmul(rzA[j][:],
                                                 s_recT[:, bass.ts(k, 128)],
                                                 w_hh0[:, col:col + 512],
                                                 start=(k == 0), stop=False)
                            col = k * G3 + 2048 + half * 512
                            nc.tensor.matmul(hnA[:], s_recT[:, bass.ts(k, 128)],
                                             w_hh0[:, col:col + 512],
                                             start=False, stop=(k == KH - 1))
                    # one-hot gather closes the rz accumulations
                    for j in range(2):
                        nc.tensor.matmul(rzA[j][:], ohT[:],
                                         m0b[:, bass.ts(g0 + j, 512)],
                                         start=(t == 0), stop=True)
                    rz[g0], rz[g0 + 1] = rzA
                    hn[half] = hnA
                xn = []
                for g in range(2):
                    ps = ps_gb.tile([128, 512], F32, tag="gb", name=f"l0xn{g}")
                    nc.tensor.matmul(ps[:], ohT[:],
                                     m0b[:, 2048 + g * 512:2048 + (g + 1) * 512],
                                     start=True, stop=True)
                    xn.append(ps)
                return rz + hn + xn
            else:
                banks = {}
                # two groups of [rz, rz, hn, xn] to bound live psum at 4
                for half in range(2):
                    rzA = [ps_gb.tile([128, 512], F32, tag="gb",
                                      name=f"l1rz{half}{j}") for j in range(2)]
                    hnA = ps_gb.tile([128, 512], F32, tag="gb", name=f"l1hn{half}")
                    xnA = ps_gb.tile([128, 512], F32, tag="gb", name=f"l1xn{half}")
                    g0 = 2 * half           # rz bank indices g0, g0+1
                    # biases open every accumulation
                    for j in range(2):
                        nc.tensor.matmul(rzA[j][:], ones_row[0:1, :],
                                         b_brz1[0:1, bass.ts(g0 + j, 512)],
                                         start=True, stop=False)
                    nc.tensor.matmul(hnA[:], ones_row[0:1, :],
                                     b_bhh1n[0:1, bass.ts(half, 512)],
                                     start=True, stop=(t == 0))
                    nc.tensor.matmul(xnA[:], ones_row[0:1, :],
                                     b_bn1[0:1, bass.ts(half, 512)],
                                     start=True, stop=False)
                    # recurrent part (gh1): rz pair + hn
                    if t > 0:
                        for k in range(KH):
                            for j in range(2):
                                col = k * G3 + (g0 + j) * 512
                                nc.tensor.matmul(rzA[j][:],
                                                 s_recT[:, bass.ts(k, 128)],
                                                 w_hh1[:, col:col + 512],
                                                 start=False, stop=False)
                            col = k * G3 + 2048 + half * 512
                            nc.tensor.matmul(hnA[:], s_recT[:, bass.ts(k, 128)],
                                             w_hh1[:, col:col + 512],
                                             start=False, stop=(k == KH - 1))
                    # input part (gx1): rz pair + xn
                    for k in range(KH):
                        for j in range(2):
                            col = k * G3 + (g0 + j) * 512
                            nc.tensor.matmul(rzA[j][:], s_inT[:, bass.ts(k, 128)],
                                             w_ih1[:, col:col + 512],
                                             start=False, stop=(k == KH - 1))
                        col = k * G3 + 2048 + half * 512
                        nc.tensor.matmul(xnA[:], s_inT[:, bass.ts(k, 128)],
                                         w_ih1[:, col:col + 512],
                                         start=False, stop=(k == KH - 1))
                    banks[f"rz{g0}"] = rzA[0]
                    banks[f"rz{g0+1}"] = rzA[1]
                    banks[f"hn{half}"] = hnA
                    banks[f"xn{half}"] = xnA
                return [banks["rz0"], banks["rz1"], banks["rz2"], banks["rz3"],
                        banks["hn0"], banks["hn1"], banks["xn0"], banks["xn1"]]

        def gates_and_update(layer, banks, s_state):
            """r,z,n gate math; updates s_state in new buffer; returns
            (s_new_f32, sT_new_bf16)."""
            L = layer
            r = pgate.tile([B, H], BF16, tag=f"r{L}")
            z = pgate.tile([B, H], BF16, tag=f"z{L}")
            tt = pgate.tile([B, H], BF16, tag=f"tA{L}")
            u = pgate.tile([B, H], BF16, tag=f"tB{L}")
            n = pgate.tile([B, H], BF16, tag=f"n{L}")
            d = pgate.tile([B, H], BF16, tag=f"tA{L}", name=f"d{L}")
            zd = pgate.tile([B, H], BF16, tag=f"tB{L}", name=f"zd{L}")
            for g in range(2):
                nc.scalar.activation(r[:, bass.ts(g, 512)], banks[g][:], AF.Sigmoid)
            for g in range(2):
                nc.scalar.activation(z[:, bass.ts(g, 512)], banks[2 + g][:], AF.Sigmoid)
            for g in range(2):
                nc.vector.tensor_tensor(tt[:, bass.ts(g, 512)],
                                        r[:, bass.ts(g, 512)], banks[4 + g][:],
                                        ALU.mult)
            for g in range(2):
                nc.vector.tensor_tensor(u[:, bass.ts(g, 512)],
                                        tt[:, bass.ts(g, 512)], banks[6 + g][:],
                                        ALU.add)
            nc.scalar.activation(n[:], u[:], AF.Tanh)
            nc.vector.tensor_tensor(d[:], s_state[:], n[:], ALU.subtract)
            nc.vector.tensor_tensor(zd[:], z[:], d[:], ALU.mult)
            s_new = pstate.tile([B, H], BF16, tag=f"s{L}")
            nc.vector.tensor_tensor(s_new[:], n[:], zd[:], ALU.add)
            # transpose 8 chunks -> sT
            tp = ps_tp.tile([128, H], BF16, tag="tp")
            for k in range(KH):
                nc.tensor.transpose(tp[:, bass.ts(k, 128)],
                                    s_new[:, bass.ts(k, 128)], ident[:])
            sT = pstate.tile([128, H], BF16, tag=f"sT{L}")
            nc.vector.tensor_copy(sT[:], tp[:])
            return s_new, sT

        for t in range(n_steps):
            # one-hot of input token column t (layout [B, V] -> transpose -> [V, B])
            oh = psoft.tile([B, V], BF16, tag="oh")
            nc.vector.tensor_scalar(oh[:], iota64[:], x_sb[:, t:t + 1], None,
                                    ALU.is_equal)
            ohps = ps_oh.tile([V, 128], BF16, tag="ohps")
            nc.tensor.transpose(ohps[:], oh[:], ident[:])
            ohT = psoft.tile([V, 128], BF16, tag="ohT")
            nc.vector.tensor_copy(ohT[:], ohps[:])

            # ---- layer 0 ----
            banks0 = gate_banks(0, t, ohT, None, s0T_prev)
            s0, s0T = gates_and_update(0, banks0, s0)
            # ---- layer 1 ----
            banks1 = gate_banks(1, t, None, s0T, s1T_prev)
            s1, s1T = gates_and_update(1, banks1, s1)
            s0T_prev, s1T_prev = s0T, s1T

            # ---- FC + log_softmax + NLL ----
            fc = ps_fc.tile([128, V], F32, tag="fc", name="fc")
            nc.tensor.matmul(fc[:, 0:V], ones_row[0:1, :], b_bfc[0:1, :],
                             start=True, stop=False)
            for k in range(KH):
                nc.tensor.matmul(fc[:, 0:V], s1T[:, bass.ts(k, 128)],
                                 w_fc[:, bass.ts(k, V)],
                                 start=False, stop=(k == KH - 1))
            m = psoft.tile([B, 1], F32, tag="m")
            nc.vector.reduce_max(m[:], fc[:, 0:V], axis=mybir.AxisListType.X)
            nm = psoft.tile([B, 1], F32, tag="nm")
            nc.vector.tensor_scalar_mul(nm[:], m[:], -1.0)
            ex = psoft.tile([B, V], F32, tag="ex")
            nc.scalar.activation(ex[:], fc[:, 0:V], AF.Exp, bias=nm[:, 0:1])
            sm = psoft.tile([B, 1], F32, tag="sm")
            nc.vector.reduce_sum(sm[:], ex[:], axis=mybir.AxisListType.X)
            ls = psoft.tile([B, 1], F32, tag="ls")
            nc.scalar.activation(ls[:], sm[:], AF.Ln)
            c = psoft.tile([B, 1], F32, tag="c")
            nc.vector.tensor_tensor(c[:], m[:], ls[:], ALU.add)
            lp = psoft.tile([B, V], F32, tag="lp")
            nc.vector.tensor_scalar(lp[:], fc[:, 0:V], c[:, 0:1], None, ALU.subtract)
            nc.sync.dma_start(d_lp[t], lp[:])
            # masked one-hot of target column t+1 (col0 masked via iota_m64)
            moh = psoft.tile([B, V], F32, tag="moh")
            nc.vector.tensor_scalar(moh[:], iota_m64[:], x_sb[:, t + 1:t + 2], None,
                                    ALU.is_equal)
            junk = psoft.tile([B, V], F32, tag="junk")
            nc.vector.tensor_tensor(junk[:], lp[:], moh[:], ALU.mult)
            tok = psoft.tile([B, 1], F32, tag="tok")
            nc.vector.reduce_sum(tok[:], junk[:], axis=mybir.AxisListType.X)
            acc_in, acc_out = nll[t % 2], nll[(t + 1) % 2]
            nc.vector.tensor_tensor(acc_out[:], acc_in[:], tok[:], ALU.subtract)

        # ---------- finale ----------
        nll_fin = nll[n_steps % 2]
        ne = pw.tile([B, S], F32, tag="ne")
        nc.vector.tensor_scalar(ne[:], x_sb[:], 0, None, ALU.not_equal)
        lens = pw.tile([B, 1], F32, tag="lens")
        nc.vector.reduce_sum(lens[:], ne[:], axis=mybir.AxisListType.X)
        inv = pw.tile([B, 1], F32, tag="inv")
        nc.vector.reciprocal(inv[:], lens[:])
        mol = pw.tile([B, 1], F32, tag="mol")
        nc.vector.tensor_tensor(mol[:], nll_fin[:], inv[:], ALU.mult)
        nc.sync.dma_start(d_mol[:], mol[:])
        lsps = ps_fc.tile([128, V], F32, tag="fc", name="lsps")
        nc.tensor.matmul(lsps[0:1, 0:1], nll_fin[:, 0:1], ones_col[:, 0:1],
                         start=True, stop=True)
        lsum = pw.tile([1, 1], F32, tag="lsum")
        nc.vector.tensor_copy(lsum[:], lsps[0:1, 0:1])
        nc.sync.dma_start(d_lsum[:], lsum[:])


# ---------------------------------------------------------------------------
# host side
# ---------------------------------------------------------------------------

def _chunked(w, kchunks):
    """[K, N] -> [128, kchunks*N] with chunk k at cols [k*N:(k+1)*N]."""
    K, Nn = w.shape
    assert K == kchunks * 128
    return np.ascontiguousarray(
        w.reshape(kchunks, 128, Nn).transpose(1, 0, 2).reshape(128, kchunks * Nn))


def _bf16(a):
    return np.asarray(a, np.float32).astype(ml_dtypes.bfloat16)


def prep_inputs(inputs):
    emb = np.asarray(inputs['emb'], np.float32)
    W_ih0 = np.asarray(inputs['W_ih0'], np.float32)
    W_hh0 = np.asarray(inputs['W_hh0'], np.float32)
    b_ih0 = np.asarray(inputs['b_ih0'], np.float32)
    b_hh0 = np.asarray(inputs['b_hh0'], np.float32)
    W_ih1 = np.asarray(inputs['W_ih1'], np.float32)
    W_hh1 = np.asarray(inputs['W_hh1'], np.float32)
    b_ih1 = np.asarray(inputs['b_ih1'], np.float32)
    b_hh1 = np.asarray(inputs['b_hh1'], np.float32)
    W_fc = np.asarray(inputs['W_fc'], np.float32)
    b_fc = np.asarray(inputs['b_fc'], np.float32)
    x = np.asarray(inputs['x'])

    brow0 = (b_ih0 + np.concatenate([b_hh0[:2 * H], np.zeros(H, np.float32)]))
    shared = {
        "embT": _bf16(_chunked(emb.T, E // 128)),
        "wih0": _bf16(_chunked(np.ascontiguousarray(W_ih0.T), E // 128)),
        "whh0": _bf16(_chunked(np.ascontiguousarray(W_hh0.T), KH)),
        "wih1": _bf16(_chunked(np.ascontiguousarray(W_ih1.T), KH)),
        "whh1": _bf16(_chunked(np.ascontiguousarray(W_hh1.T), KH)),
        "wfc": _bf16(_chunked(np.ascontiguousarray(W_fc.T), KH)),
        "brow0": _bf16(brow0)[None, :],
        "bhh0n": _bf16(b_hh0[2 * H:])[None, :],
        "brz1": _bf16((b_ih1 + b_hh1)[:2 * H])[None, :],
        "bn1": _bf16(b_ih1[2 * H:])[None, :],
        "bhh1n": _bf16(b_hh1[2 * H:])[None, :],
        "bfc": _bf16(b_fc)[None, :],
    }
    in_maps = []
    for c in range(NC):
        m = dict(shared)
        m["x"] = np.ascontiguousarray(x[c * B:(c + 1) * B].astype(np.float32))
        in_maps.append(m)
    return in_maps


def assemble_outputs(results):
    lp = np.concatenate(
        [r["lp"].transpose(1, 0, 2)[None] for r in results], axis=0
    ).reshape(N, T, V)
    mol = np.concatenate([r["mol"][:, 0] for r in results])
    loss = np.float32(sum(float(r["lsum"][0, 0]) for r in results) / N)
    return lp.astype(np.float32), mol.astype(np.float32), loss


_NC_CACHE = {}


def kernel(**inputs):
    key = "full"
    if key not in _NC_CACHE:
        _NC_CACHE[key] = build_kernel(T)
    nc = _NC_CACHE[key]
    in_maps = prep_inputs(inputs)
    res = run_bass_kernel_spmd(nc, in_maps, list(range(NC)))
    return assemble_outputs(res.results)
